# revision 2
# baseline (speedup 1.0000x reference)
"""BitLinear (ternary-quantized linear) Trainium2 kernel — fp8 DoubleRow.

out = (x @ ternary_quantize(W).T) * mean(|W|),  alpha = 0.7

Sharding: tensor-parallel over out_features (8192 -> 8 x 1024). Each core
gets all of x and its own weight shard.

The matmul runs in fp8-e4m3 DoubleRow mode (2 MACs per PE cell per
cycle, ~2x the bf16 tensor peak): wq in {-1,0,+1} is exact in fp8, and x
is quantized to e4m3 on the host. Pure e4m3 x measures rel err 1.93e-2
against the fp32 reference — too close to the 2e-2 gate — so the last
K_SPLIT=256 k-columns are carried as hi+lo e4m3 plane pairs (their
contribution exact to ~1e-4), which measures 1.81e-2 end to end. Slot
count S = K + K_SPLIT = 2304 -> 18 fp8 planes -> 9 DoubleRow k-pair
matmuls per psum tile; tensor time = (S/2)/2048 of the bf16 437us floor
~= 246us, and the measured body sits at ~250us.

Host packing is data movement + dtype encode only (no reference math):
  full8 = vstack(e4m3(xT), e4m3(xT[K_PURE:] - fp32(e4m3(xT[K_PURE:]))))
  xq[p, plane, t]  = full8[p*NPLANES + plane, t]
  wTp[p, plane, o] = w_shard.T[karr[p*NPLANES + plane], o],
                     karr = [0..K) ++ [K_PURE..K)
Split columns appear twice in wTp (hi and lo slots multiply the same
weight), so the device quantizer and matmul loops are completely uniform.

weight_scale = mean(|W|) needs a global sum of |w| across the 8 shards;
an in-launch AllReduce costs ~150us here, so it stays the baseline's
two-launch scheme: launch 1 computes per-core partial |w| sums (device
reduce tree kept BYTE-IDENTICAL to the baseline: the closest |w| to the
quantization threshold is 1.1e-8 relative away, and any change to the
summation tree risks flipping a ternary decision vs the reference);
the host concatenates the 8 [128,1] vectors (pure data movement) and
launch 2 sums them on device.

Main launch per core:
  phase 0: threshold from gsums (reduce + gpsimd partition_all_reduce)
  prewarm: dummy DoubleRow matmuls during the head keep the PE busy so
    the HAM clock-gate opens before the real matmuls arrive
  phase 1: quantize wTp fp32 -> wq fp8 {-1,0,+1} [128, NPLANES, 1024],
    pipelined with its own weight DMA stream on the sync queue
  phase 2: per 2048-token chunk (prefetched on the gpsimd/SWDGE queue so
    it never queues behind the quantizer's weight DMAs): for each o-tile
    of 128: 9 k-pair DoubleRow matmuls per 512-token psum bank, k-pair
    innermost (measured faster than hoisting the stationary operand:
    changing stationaries lets the PE pull the next LDWEIGHTS ahead),
    4 banks accumulating + 4 draining; scale by mean(|W|) on the scalar
    engine during PSUM->SBUF into a [128, 2048] fp16 staging tile, one
    output DMA per o-tile/chunk with 4KB contiguous per-partition lines,
    out in [o, t] layout (host transposes back).
"""

import numpy as np
import ml_dtypes

import concourse.mybir as mybir
import concourse.tile as tile
from concourse import bacc, bass_isa
from concourse.bass_utils import run_bass_kernel_spmd

N_CORES = 8
B, S, IN_F, OUT_F = 4, 2048, 2048, 8192
T_FULL = B * S              # 8192 tokens
K = IN_F                    # contraction dim
OS = OUT_F // N_CORES       # 1024 out-features per core
P = 128
KT = K // P                 # 16 k-tiles
ALPHA = 0.7
N_TOTAL = float(OUT_F * K)  # 2**24, so 1/N_TOTAL is exact in fp32

K_SPLIT = 256               # k-columns carried as exact hi+lo fp8 pairs
K_PURE = K - K_SPLIT
NPLANES = (K + K_SPLIT) // P  # 18 fp8 slot planes
KP = NPLANES // 2             # 9 DoubleRow k-pairs

C = 2048                    # tokens per x DMA chunk
TS = 512                    # moving tokens per matmul (= one PSUM bank)
NF = 256                    # quantizer o-chunk

F8 = mybir.dt.float8e4
NP_F8 = ml_dtypes.float8_e4m3

LAST_RESULTS = None         # test harness peeks at exec_time_ns here
_PROGRAMS = {}              # compiled program cache across kernel() calls


def _build_phase_a(loops=1, barrier=False):
    """Tiny first launch: per-core sum of |w shard| -> [128, 1] output.

    Byte-identical reduce tree to the bf16 baseline (threshold
    bit-exactness vs the reference).
    """
    F32 = mybir.dt.float32
    AX = mybir.AxisListType.X
    Alu = mybir.AluOpType
    nc = bacc.Bacc(
        "TRN2", target_bir_lowering=False, debug=False, num_devices=N_CORES
    )
    wT = nc.dram_tensor("wT", [K, OS], F32, kind="ExternalInput").ap()
    asum_out = nc.dram_tensor("asum", [P, 1], F32, kind="ExternalOutput").ap()
    with tile.TileContext(nc) as tc:
        for _loop in range(loops):
            if barrier and _loop > 0:
                tc.strict_bb_all_engine_barrier()
            with (
                tc.tile_pool(name="wpa", bufs=3) as wpa,
                tc.tile_pool(name="cpa", bufs=1) as cpa,
            ):
                wT_t = wT.rearrange("(n p) o -> p n o", p=P)
                KG = 2
                asum = cpa.tile([P, KT // KG], F32)
                for g in range(KT // KG):
                    wt = wpa.tile([P, KG, OS], F32, name="wt", tag="wt")
                    nc.sync.dma_start(wt[:], wT_t[:, g * KG : (g + 1) * KG, :])
                    nc.vector.tensor_reduce(
                        asum[:, g : g + 1], wt[:],
                        axis=mybir.AxisListType.XY, op=Alu.add,
                        apply_absolute_value=True,
                    )
                asum1 = cpa.tile([P, 1], F32)
                nc.vector.tensor_reduce(asum1[:], asum[:], axis=AX, op=Alu.add)
                nc.sync.dma_start(asum_out[:], asum1[:])
    nc.compile()
    return nc


def _build_program(loops=1, barrier=False):
    F32 = mybir.dt.float32
    F16 = mybir.dt.float16
    AX = mybir.AxisListType.X
    Alu = mybir.AluOpType
    DR = mybir.MatmulPerfMode.DoubleRow

    nc = bacc.Bacc(
        "TRN2", target_bir_lowering=False, debug=False, num_devices=N_CORES
    )
    xq = nc.dram_tensor("xq", [P, KP, 2, T_FULL], F8, kind="ExternalInput").ap()
    wTp = nc.dram_tensor("wTp", [P, NPLANES, OS], F32, kind="ExternalInput").ap()
    gsums = nc.dram_tensor("gsums", [P, N_CORES], F32, kind="ExternalInput").ap()
    out = nc.dram_tensor("out", [OS, T_FULL], F16, kind="ExternalOutput").ap()

    with tile.TileContext(nc) as tc:
        for _loop in range(loops):
            if barrier and _loop > 0:
                tc.strict_bb_all_engine_barrier()
            with (  # noqa: SIM117
                tc.tile_pool(name="wfp", bufs=4) as wfp,
                tc.tile_pool(name="wqp", bufs=1) as wqp,
                tc.tile_pool(name="cpool", bufs=1) as cpool,
                tc.tile_pool(name="xpool", bufs=2) as xpool,
                tc.tile_pool(name="opool", bufs=4) as opool,
                tc.tile_pool(name="psum", bufs=2, space="PSUM") as psum_pool,
            ):
                # ---- phase 0: threshold from gsums ----
                gsum8 = cpool.tile([P, N_CORES], F32)
                nc.sync.dma_start(gsum8[:], gsums[:])
                gsum = cpool.tile([P, 1], F32)
                nc.vector.tensor_reduce(gsum[:], gsum8[:], axis=AX, op=Alu.add)
                tot = cpool.tile([P, 1], F32)
                nc.gpsimd.partition_all_reduce(
                    tot[:], gsum[:], channels=P,
                    reduce_op=bass_isa.ReduceOp.add,
                )
                # mean = tot * 2**-24 (exact); thr = 0.7 * mean
                mean_t = cpool.tile([P, 1], F32)
                nc.vector.tensor_scalar_mul(mean_t[:], tot[:], 1.0 / N_TOTAL)
                thr_t = cpool.tile([P, 1], F32)
                nc.vector.tensor_scalar_mul(thr_t[:], mean_t[:], ALPHA)
                nthr_t = cpool.tile([P, 1], F32)
                nc.vector.tensor_scalar_mul(nthr_t[:], thr_t[:], -1.0)

                # prefetch x chunk 0 on the gpsimd (SWDGE) queue in TS-sized
                # sub-DMAs: doesn't queue behind the quantizer's wf DMAs on
                # the sync queue, and matmul j only waits for sub-DMA j
                xb0 = xpool.tile([P, KP, 2, C], F8, name="xb", tag="xb")
                for j in range(C // TS):
                    nc.gpsimd.dma_start(
                        xb0[:, :, :, j * TS : (j + 1) * TS],
                        xq[:, :, :, j * TS : (j + 1) * TS],
                    )

                # tensor-engine prewarm: keep the PE busy during the quant
                # head so the HAM clock-gate opens (~3.4us busy window)
                # before the real matmuls arrive
                dumw = cpool.tile([P, 2, P], F8)
                dumx = cpool.tile([P, 2, TS], F8)
                nc.vector.memset(dumw[:], 1.0)
                nc.vector.memset(dumx[:], 1.0)
                pw = psum_pool.tile([P, TS], F32, name="po0")
                for _r in range(40):
                    nc.tensor.matmul(
                        pw[:], dumw[:], dumx[:], start=True, stop=True,
                        perf_mode=DR,
                    )

                # ---- phase 1: quantize wTp fp32 -> wq fp8 {-1,0,+1} ----
                wq = wqp.tile([P, NPLANES, OS], F8)
                for oc in range(OS // NF):
                    osl = slice(oc * NF, (oc + 1) * NF)
                    for pl in range(NPLANES):
                        wf = wfp.tile([P, NF], F32, name="wf", tag="wf")
                        nc.sync.dma_start(wf[:], wTp[:, pl, osl])
                        # neg = (w <= -thr) in {0,1}
                        neg = wfp.tile([P, NF], F16, name="neg", tag="neg")
                        nc.vector.tensor_scalar(
                            neg[:], wf[:], nthr_t[:], None, op0=Alu.is_le
                        )
                        # wq = (w >= thr) - neg  in {-1, 0, 1}
                        nc.vector.scalar_tensor_tensor(
                            wq[:, pl, osl], wf[:], thr_t[:], neg[:],
                            op0=Alu.is_ge, op1=Alu.subtract,
                        )

                # ---- phase 2: DoubleRow matmul sweep over token chunks ----
                for tch in range(T_FULL // C):
                    if tch == 0:
                        xb = xb0
                    else:
                        xb = xpool.tile([P, KP, 2, C], F8, name="xb", tag="xb")
                        nc.gpsimd.dma_start(
                            xb[:], xq[:, :, :, tch * C : (tch + 1) * C]
                        )
                    for ot in range(OS // P):
                        osl = slice(ot * P, (ot + 1) * P)
                        pos = [
                            psum_pool.tile([P, TS], F32, name=f"po{j}")
                            for j in range(C // TS)
                        ]
                        for j in range(C // TS):
                            for kp in range(KP):
                                nc.tensor.matmul(
                                    pos[j][:],
                                    wq[:, 2 * kp : 2 * kp + 2, osl],
                                    xb[:, kp, :, j * TS : (j + 1) * TS],
                                    start=(kp == 0),
                                    stop=(kp == KP - 1),
                                    perf_mode=DR,
                                )
                        # scale by mean(|W|) on the scalar engine during
                        # PSUM->SBUF into fp16 staging; one DMA per o-tile
                        # with 4KB contiguous per-partition lines
                        ob = opool.tile([P, C], F16, name="ob", tag="ob")
                        for j in range(C // TS):
                            nc.scalar.mul(
                                ob[:, j * TS : (j + 1) * TS], pos[j][:],
                                mean_t[:],
                            )
                        nc.sync.dma_start(
                            out[osl, tch * C : (tch + 1) * C], ob[:]
                        )
    nc.compile()
    return nc


_KARR = np.concatenate([np.arange(K), np.arange(K_PURE, K)])


def _pack_x(x):
    """x [B,S,K] fp32 -> xq [P, KP, 2, T] e4m3 (hi planes + split lo planes)."""
    xT = np.ascontiguousarray(x.reshape(T_FULL, K).T)  # [K, T]
    hi8 = xT.astype(NP_F8)
    lo8 = (xT[K_PURE:] - hi8[K_PURE:].astype(np.float32)).astype(NP_F8)
    full8 = np.concatenate([hi8, lo8], axis=0)  # [K + K_SPLIT, T]
    return np.ascontiguousarray(full8).reshape(P, KP, 2, T_FULL)


def _pack_w(shard):
    """w shard [OS, K] fp32 -> wTp [P, NPLANES, OS] fp32 (row-gathered)."""
    wT = np.ascontiguousarray(shard.T)  # [K, OS]
    return np.ascontiguousarray(wT[_KARR]).reshape(P, NPLANES, OS)


def kernel(x, weight):
    global LAST_RESULTS
    x = np.asarray(x, dtype=np.float32)
    weight = np.asarray(weight, dtype=np.float32)
    assert x.shape == (B, S, IN_F), x.shape
    assert weight.shape == (OUT_F, IN_F), weight.shape

    xq = _pack_x(x)
    in_maps = []
    for c in range(N_CORES):
        shard = weight[c * OS : (c + 1) * OS, :]
        in_maps.append(
            {
                "xq": xq,
                "wTp": _pack_w(shard),
                "wT": np.ascontiguousarray(shard.T),
            }
        )

    cores = list(range(N_CORES))
    # launch 1: per-core partial |w| sums (all math on device)
    if "a" not in _PROGRAMS:
        _PROGRAMS["a"] = _build_phase_a()
    res_a = run_bass_kernel_spmd(_PROGRAMS["a"], in_maps, cores)
    gs = np.concatenate(  # pure data movement, no host math
        [res_a.results[c]["asum"] for c in range(N_CORES)], axis=1
    )
    for m in in_maps:
        m["gsums"] = gs

    if "main" not in _PROGRAMS:
        _PROGRAMS["main"] = _build_program()
    res = run_bass_kernel_spmd(_PROGRAMS["main"], in_maps, cores)
    LAST_RESULTS = res
    outs = [res.results[c]["out"] for c in range(N_CORES)]  # [OS, T] fp16
    full = np.concatenate(outs, axis=0)  # [OUT_F, T]
    return np.ascontiguousarray(full.T.astype(np.float32)).reshape(B, S, OUT_F)
